# revision 19
# baseline (speedup 1.0000x reference)
"""Sparse multi-head self-attention (sliding window + global columns) on 8
Trainium2 NeuronCores.

Sharding: fully data-parallel over the sequence dimension. Core c produces
output rows [512c, 512c+512). Each core recomputes k/v for a 128-row halo on
each side of its slice plus the 16 global key rows (j % 256 == 0), so no
collectives are needed.

v4 design:
- all matmul operands bf16 (fp32 PSUM accumulation): full-rate PE, FWL
  weight loads, half DMA/SBUF
- all weights preloaded to SBUF; input-x DMAs ride the scalar HWDGE queue so
  they don't sit behind the 7 MB weight preload on the sync queue
- global key rows appended to the local band (one projection pass, 784 cols)
- band score tiles paired into 4 uniform [128,384] super-tiles per head:
  one exp + one mask-multiply per super-tile instead of 6
- v stored per head-pair [128, 7, 130] with a ones column per head: sums
  ride the AV matmul; single strided DVE copy per transposed v tile
- softmax reciprocal: exp(-ln s) seed on ACT + one fused Newton step on DVE,
  broadcast via tiny (-ones) matmul; raw o evicted early to free PSUM
"""
import sys

sys.path.insert(0, "/opt/trn_rl_repo")

import numpy as np
import concourse.bass as bass
import concourse.mybir as mybir
from concourse.tile import TileContext

# ---------------------------------------------------------------- constants
B, T, D = 1, 4096, 1024
H, HD = 16, 64
W = 128
GSTRIDE = 256
ROPE_BASE = 10000.0
NCORES = 8
TLOC = T // NCORES            # 512 own rows per core
HALO = 128
NL = TLOC + 2 * HALO          # 768 rows incl. halo
NG = T // GSTRIDE             # 16 global keys
NLG = NL + NG                 # 784 = local band + appended global rows
NPAD = 896                    # 7*128, zero-padded
NRT = NPAD // 128             # 7 row tiles through RoPE
NT = NL // 128                # 6 local 128-row key tiles
NQB = TLOC // 128             # 4 query blocks per core
NDT = D // 128                # 8 din tiles
SCALE = 1.0 / np.sqrt(HD)

# per-m query-column ranges within the core's 512 own rows
QS = [0, 0, 0, 128, 256, 384]
QW = [128, 256, 384, 384, 256, 128]
# band tiles paired into uniform [128,384] super-tiles: m -> (super, col)
SUP = [(0, 0), (0, 128), (1, 0), (2, 0), (3, 0), (3, 256)]
NSUP = 4
SUPW = 384

DT = mybir.dt.float32
BF = mybir.dt.bfloat16
F32R = mybir.dt.float32r
FT = mybir.ActivationFunctionType

_cache = {}


# ------------------------------------------------------- walrus workaround
def _fix_multi_waits(nc):
    """This walrus build encodes at most ONE sem wait per instruction; hoist
    extra waits onto same-engine NoOps inserted just before the owner."""
    count = 0
    for fn in nc.m.functions:
        for bb in fn.blocks:
            old = bb.instructions
            if not any(
                i.sync_info is not None and len(i.sync_info.on_wait or []) > 1
                for i in old
            ):
                continue
            new = []
            for inst in old:
                si = inst.sync_info
                waits = list(si.on_wait) if si is not None and si.on_wait else []
                if len(waits) > 1:
                    for w in waits[:-1]:
                        count += 1
                        new.append(
                            mybir.InstNoOp(
                                name=f"I-waitfix-{count}",
                                engine=inst.engine,
                                bass_nofuse=True,
                                sync_info=mybir.SyncInfo(on_wait=[w], on_update=[]),
                            )
                        )
                    inst.sync_info = mybir.SyncInfo(
                        on_wait=[waits[-1]], on_update=list(si.on_update or [])
                    )
                new.append(inst)
            bb.instructions = new
    return count


def _bcast_mid(ap2d, reps):
    """[P, F] AP -> [P, reps, F] AP broadcasting along a middle free dim."""
    a = [list(x) for x in ap2d.ap]
    return bass.AP(tensor=ap2d.tensor, offset=ap2d.offset,
                   ap=[a[0], [0, reps], a[1]])


# ------------------------------------------------------------ bass program
def build_program():
    nc = bass.Bass()

    xlg = nc.dram_tensor("xlg", [NPAD, D], BF, kind="ExternalInput")
    cslg = nc.dram_tensor("cslg", [NPAD, D], BF, kind="ExternalInput")
    wts = nc.dram_tensor("wts", [128, 24, NDT, 128], BF, kind="ExternalInput")
    wos = nc.dram_tensor("wos", [128, 2, NDT, 512], BF, kind="ExternalInput")
    ball = nc.dram_tensor("ball", [128, 24], DT, kind="ExternalInput")
    bo = nc.dram_tensor("bo", [1, D], F32R, kind="ExternalInput")
    mloc = nc.dram_tensor("mloc", [NSUP, 128, SUPW], BF, kind="ExternalInput")
    mglob = nc.dram_tensor("mglob", [NG, TLOC], BF, kind="ExternalInput")
    identd = nc.dram_tensor("identd", [128, 128], BF, kind="ExternalInput")
    out = nc.dram_tensor("out", [TLOC, D], DT, kind="ExternalOutput")

    with TileContext(nc) as tc:
        _build_body(nc, tc, xlg, cslg, wts, wos, ball, bo, mloc, mglob,
                    identd, out)
    _fix_multi_waits(nc)
    return nc


def _build_body(nc, tc, xlg, cslg, wts, wos, ball, bo, mloc, mglob,
                identd, out):
    from contextlib import ExitStack
    ctx = ExitStack()
    with ctx:
        singles = ctx.enter_context(tc.tile_pool(name="singles", bufs=1))
        vtpool = ctx.enter_context(tc.tile_pool(name="vtpool", bufs=2))
        pepool = ctx.enter_context(tc.tile_pool(name="pepool", bufs=3))
        sden = ctx.enter_context(tc.tile_pool(name="sden", bufs=2))
        bcpool = ctx.enter_context(tc.tile_pool(name="bcpool", bufs=2))
        ostpool = ctx.enter_context(tc.tile_pool(name="ostpool", bufs=2))
        sopool = ctx.enter_context(tc.tile_pool(name="sopool", bufs=2))
        ps_p = ctx.enter_context(tc.tile_pool(name="ps_p", bufs=2, space="PSUM"))
        ps_s = ctx.enter_context(tc.tile_pool(name="ps_s", bufs=3, space="PSUM"))
        ps_po = ctx.enter_context(tc.tile_pool(name="ps_po", bufs=2, space="PSUM"))
        ps_pb = ctx.enter_context(tc.tile_pool(name="ps_pb", bufs=1, space="PSUM"))

        # ---------------- constants / weights, all preloaded up front.
        # Weights ride the sync queue; x/cos-sin/ident ride the scalar queue
        # so the rope phase isn't stuck behind 7 MB of weights.
        ident = singles.tile([128, 128], BF)
        wts_sb = singles.tile([128, 24, NDT, 128], BF)
        for hp in range(NDT):
            for c in (hp, 8 + hp, 16 + hp):
                nc.sync.dma_start(wts_sb[:, c], wts[:, c])
        wos_sb = singles.tile([128, 2, NDT, 512], BF)
        for chp in range(2):
            nc.sync.dma_start(wos_sb[:, chp], wos[:, chp])
        ball_sb = singles.tile([128, 24], DT)
        nc.sync.dma_start(ball_sb[:], ball[:])
        bo_sb = singles.tile([1, D], F32R)
        nc.sync.dma_start(bo_sb[:], bo[:])
        ones = singles.tile([1, 128], F32R)
        nc.vector.memset(ones[:].bitcast(DT), 1.0)
        negones = singles.tile([1, 64], F32R)
        nc.vector.memset(negones[:].bitcast(DT), -1.0)
        mask_sb = []
        for s in range(NSUP):
            t = singles.tile([128, SUPW], BF, tag=f"mask{s}", name=f"mask{s}")
            nc.sync.dma_start(t[:], mloc[s])
            mask_sb.append(t)
        mg_sb = singles.tile([NG, TLOC], BF)
        nc.sync.dma_start(mg_sb[:], mglob[:])


        # persistent tensors
        xTa = singles.tile([128, NDT, NPAD], BF)
        qT = [singles.tile([128, TLOC], BF, tag=f"qT{c}", name=f"qT{c}")
              for c in range(NDT)]
        kT = [singles.tile([128, NLG], BF, tag=f"kT{c}", name=f"kT{c}")
              for c in range(NDT)]
        # v per head-pair, natural orientation: [keys, m, 130]
        # head A cols 0:64 (+ones at 64), head B cols 65:129 (+ones at 129)
        vp = [singles.tile([128, NRT, 130], BF, tag=f"vp{c}", name=f"vp{c}")
              for c in range(NDT)]
        for c in range(NDT):
            v4 = vp[c][:].rearrange("p m (b f) -> p m b f", b=2)
            nc.vector.memset(v4[:, :, :, HD:HD + 1], 1.0)
        oT = [singles.tile([128, TLOC], BF, tag=f"oT{k}", name=f"oT{k}")
              for k in range(NDT)]

        # ---------------- RoPE (fp32, bf16 out) + PE transpose, paired evict
        with tc.tile_pool(name="ropepool", bufs=2) as rp:
            for i in range(NRT):
                x_sb = rp.tile([128, D], BF, tag="x", name=f"x{i}")
                nc.scalar.dma_start(x_sb[:], xlg[i * 128:(i + 1) * 128, :])
                cs_sb = rp.tile([128, D], BF, tag="cs", name=f"cs{i}")
                nc.scalar.dma_start(cs_sb[:], cslg[i * 128:(i + 1) * 128, :])
                if i == 0:
                    nc.scalar.dma_start(ident[:], identd[:])
                tmp = rp.tile([128, D], BF, tag="tmp", name=f"tmp{i}")
                tmp2 = rp.tile([128, D], BF, tag="tmp2", name=f"tmp2{i}")
                roped = rp.tile([128, D], BF, tag="roped", name=f"roped{i}")
                HF = D // 2
                xe = x_sb[:, 0:HF]
                xo = x_sb[:, HF:D]
                cosb = cs_sb[:, 0:HF]
                sinb = cs_sb[:, HF:D]
                nc.vector.tensor_mul(tmp[:, 0:HF], xe, cosb)
                nc.vector.tensor_mul(tmp[:, HF:D], xo, sinb)
                nc.vector.tensor_sub(roped[:, 0:HF], tmp[:, 0:HF],
                                     tmp[:, HF:D])
                nc.vector.tensor_mul(tmp2[:, 0:HF], xe, sinb)
                nc.vector.tensor_mul(tmp2[:, HF:D], xo, cosb)
                nc.vector.tensor_add(roped[:, HF:D], tmp2[:, 0:HF],
                                     tmp2[:, HF:D])
                for kp in range(NDT // 2):
                    ptr = ps_s.tile([128, 256], BF, tag="s",
                                    name=f"ptr{i}_{kp}")
                    for j in (0, 1):
                        k = 2 * kp + j
                        nc.tensor.matmul(ptr[:, 128 * j:128 * j + 128],
                                         roped[:, k * 128:(k + 1) * 128],
                                         ident[:], is_transpose=True,
                                         start=(j == 0), stop=(j == 1))
                    dst = xTa[:, 2 * kp:2 * kp + 2, i * 128:(i + 1) * 128]
                    src = ptr[:].rearrange("p (b f) -> p b f", b=2)
                    if kp % 2 == 0:
                        nc.vector.tensor_copy(dst, src)
                    else:
                        nc.scalar.copy(dst, src)

        # ---------------- main pipeline: per head-pair hp
        for hp in range(NDT):
            # --- q projection (own 512 rows)
            pq = ps_p.tile([128, 512], DT, tag="p", name=f"pq{hp}")
            for k in range(NDT):
                nc.tensor.matmul(pq[:], wts_sb[:, hp, k, :],
                                 xTa[:, k, HALO:HALO + TLOC],
                                 start=(k == 0), stop=(k == NDT - 1))
            nc.vector.tensor_scalar_add(qT[hp][:], pq[:],
                                         ball_sb[:, hp:hp + 1])

            # --- k / v projections (784 = 768 band + 16 global rows)
            for c0, c1 in ((0, 512), (512, NLG)):
                w = c1 - c0
                pk = ps_p.tile([128, 512], DT, tag="p", name=f"pk{hp}_{c0}")
                for k in range(NDT):
                    nc.tensor.matmul(pk[:, 0:w], wts_sb[:, 8 + hp, k, :],
                                     xTa[:, k, c0:c1],
                                     start=(k == 0), stop=(k == NDT - 1))
                nc.vector.tensor_scalar_add(kT[hp][:, c0:c1], pk[:, 0:w],
                                             ball_sb[:, 8 + hp:9 + hp])
            vT = vtpool.tile([128, NPAD], BF, tag="vT", name=f"vT{hp}")
            for c0, c1 in ((0, 512), (512, NLG)):
                w = c1 - c0
                pv = ps_p.tile([128, 512], DT, tag="p", name=f"pv{hp}_{c0}")
                for k in range(NDT):
                    nc.tensor.matmul(pv[:, 0:w], wts_sb[:, 16 + hp, k, :],
                                     xTa[:, k, c0:c1],
                                     start=(k == 0), stop=(k == NDT - 1))
                nc.scalar.add(vT[:, c0:c1], pv[:, 0:w],
                              ball_sb[:, 16 + hp:17 + hp])
            nc.vector.memset(vT[:, NLG:NPAD], 0.0)
            # v -> natural orientation: PE transpose + one strided DVE copy
            for m in range(NRT):
                ptv = ps_s.tile([128, 128], BF, tag="s", name=f"ptv{hp}_{m}")
                nc.tensor.transpose(ptv[:],
                                    vT[:, m * 128:(m + 1) * 128], ident[:])
                dst = vp[hp][:, m, :].rearrange("p (b f) -> p b f", b=2)
                src = ptv[:].rearrange("p (b f) -> p b f", b=2)
                nc.vector.tensor_copy(dst[:, :, 0:HD], src[:])

            # --- attention for the two heads of this pair (interleaved)
            po = [ps_po.tile([65, TLOC], DT, tag="po", name=f"po{2 * hp + e}")
                  for e in (0, 1)]
            vpr = vp[hp][:].rearrange("p m (b f) -> p m b f", b=2)
            for s in range(NSUP):
                mem = [m for m in range(NT) if SUP[m][0] == s]
                psc, pe2 = [], []
                for e in (0, 1):
                    h = 2 * hp + e
                    off = 64 * e
                    ps_ = ps_s.tile([128, 512], DT, tag="s",
                                    name=f"psc{h}_{s}")
                    for j, m in enumerate(mem):
                        col, w, qs = SUP[m][1], QW[m], QS[m]
                        nc.tensor.matmul(ps_[:, col:col + w],
                                         kT[hp][off:off + 64,
                                                m * 128:(m + 1) * 128],
                                         qT[hp][off:off + 64, qs:qs + w],
                                         start=(j == 0),
                                         stop=(j == len(mem) - 1))
                    psc.append(ps_)
                for e in (0, 1):
                    h = 2 * hp + e
                    pe_ = pepool.tile([128, SUPW], BF, tag="pe",
                                      name=f"pe{h}_{s}")
                    nc.scalar.activation(pe_[:], psc[e][:, 0:SUPW], FT.Exp)
                    p2 = pepool.tile([128, SUPW], BF, tag="pe2",
                                     name=f"pe2{h}_{s}")
                    nc.gpsimd.tensor_mul(p2[:], pe_[:], mask_sb[s][:])
                    pe2.append(p2)
                for e in (0, 1):
                    for m in mem:
                        col, w, qs = SUP[m][1], QW[m], QS[m]
                        nc.tensor.matmul(po[e][:, qs:qs + w],
                                         vpr[:, m, e, :],
                                         pe2[e][:, col:col + w],
                                         start=(m == 0), stop=False)
            # global keys
            for e in (0, 1):
                h = 2 * hp + e
                off = 64 * e
                psg = ps_s.tile([128, 512], DT, tag="s", name=f"psg{h}")
                nc.tensor.matmul(psg[0:NG, :], kT[hp][off:off + 64, NL:NLG],
                                 qT[hp][off:off + 64, :],
                                 start=True, stop=True)
                peg = pepool.tile([NG, 512], BF, tag="peg",
                                  name=f"peg{h}", bufs=2)
                nc.scalar.activation(peg[:], psg[0:NG, :], FT.Exp)
                peg2 = pepool.tile([NG, 512], BF, tag="peg2",
                                   name=f"peg2{h}", bufs=2)
                nc.gpsimd.tensor_mul(peg2[:], peg[:], mg_sb[:])
                nc.tensor.matmul(po[e][:], vpr[0:NG, NT, e, :], peg2[:],
                                 start=False, stop=True)
            # evict raw o + sums (frees the PSUM bank), then softmax
            # denominators: 1/s = Newton step on an exp(-ln s) seed
            for e in (0, 1):
                h = 2 * hp + e
                praw = bcpool.tile([64, TLOC], DT, tag="praw",
                                   name=f"praw{h}")
                nc.vector.tensor_copy(praw[:], po[e][0:64, :])
                s_sb = sden.tile([1, TLOC], DT, tag="s_sb", name=f"ssb{h}")
                nc.vector.tensor_copy(s_sb[:], po[e][64:65, :])
                lns = sden.tile([1, TLOC], DT, tag="lns", name=f"lns{h}")
                nc.scalar.activation(lns[:], s_sb[:], FT.Ln)
                r0 = sden.tile([1, TLOC], DT, tag="r0", name=f"r0{h}")
                nc.scalar.activation(r0[:], lns[:], FT.Exp, scale=-1.0)
                u = sden.tile([1, TLOC], DT, tag="u", name=f"u{h}")
                nc.vector.tensor_mul(u[:], s_sb[:], r0[:])
                nr = sden.tile([1, TLOC], F32R, tag="nr", name=f"nr{h}")
                nc.vector.scalar_tensor_tensor(
                    nr[:], u[:], 2.0, r0[:],
                    mybir.AluOpType.subtract, mybir.AluOpType.mult)
                pb = ps_pb.tile([64, TLOC], DT, tag="pb", name=f"pb{h}")
                nc.tensor.matmul(pb[:], negones[:], nr[:],
                                 start=True, stop=True)
                if e == 0:
                    nc.vector.tensor_mul(oT[hp][0:64, :], praw[:], pb[:])
                else:
                    ost = ostpool.tile([64, TLOC], BF, tag="ost",
                                       name=f"ost{h}")
                    nc.vector.tensor_mul(ost[:], praw[:], pb[:])
                    nc.sync.dma_start(oT[hp][64:128, :], ost[:])

        # ---------------- output projection (weights already in SBUF)
        for qb in range(NQB):
            for chp in range(2):
                pout = ps_p.tile([128, 512], DT, tag="p",
                                 name=f"pout{qb}_{chp}")
                for k in range(NDT):
                    nc.tensor.matmul(pout[:],
                                     oT[k][:, qb * 128:(qb + 1) * 128],
                                     wos_sb[:, chp, k, :],
                                     start=(k == 0), stop=False)
                nc.tensor.matmul(pout[:], ones[:, 0:128],
                                 bo_sb[:, chp * 512:(chp + 1) * 512],
                                 start=False, stop=True)
                so = sopool.tile([128, 512], DT, tag="so",
                                 name=f"so{qb}_{chp}")
                nc.vector.tensor_copy(so[:], pout[:])
                nc.sync.dma_start(
                    out[qb * 128:(qb + 1) * 128, chp * 512:(chp + 1) * 512],
                    so[:])


# ------------------------------------------------------------ host helpers
def _perm():
    # all even rotation dims first (cols 0:512), then all odd (512:1024)
    p = np.arange(D).reshape(H * 32, 2)
    return np.concatenate([p[:, 0], p[:, 1]]).reshape(-1)


def _cos_sin(trows):
    """Tables matching the reference's quirky emb[..., ::2] indexing."""
    inv_freq = (1.0 / (ROPE_BASE ** (np.arange(0, HD, 2, dtype=np.float32) / HD))
                ).astype(np.float32)
    pos = trows.astype(np.float32)
    freqs = pos[:, None] * inv_freq[None, :]
    emb = np.concatenate([freqs, freqs], axis=-1)[:, ::2]      # (n, 32)
    return np.cos(emb).astype(np.float32), np.sin(emb).astype(np.float32)


def _allowed(i, j):
    ok = (np.abs(i - j) <= W) | (j % GSTRIDE == 0) | (j == 0)
    return ok & (j >= 0) & (j < T)


def make_in_maps(x, in_proj_w, in_proj_b, out_w, out_b):
    import ml_dtypes
    bf16 = ml_dtypes.bfloat16

    perm = _perm()
    x2 = np.ascontiguousarray(np.asarray(x, np.float32).reshape(T, D)[:, perm])
    wp = np.asarray(in_proj_w, np.float32)[:, perm]
    wt_full = np.ascontiguousarray(wp.T).astype(np.float32)     # (D, 3D)
    wt_full[:, 0:D] *= SCALE
    # wts[p, c, k, f] = wt_full[128k+p, 128c+f]
    wts = np.ascontiguousarray(
        wt_full.reshape(NDT, 128, 24, 128).transpose(1, 2, 0, 3)).astype(bf16)
    b = np.asarray(in_proj_b, np.float32).copy()
    b[0:D] *= SCALE
    ball = np.ascontiguousarray(b.reshape(24, 128).T)           # (128, 24)
    wo_full = np.ascontiguousarray(np.asarray(out_w, np.float32).T)  # (din,dout)
    # wos[p, chp, k, f] = wo_full[128k+p, 512chp+f]
    wos = np.ascontiguousarray(
        wo_full.reshape(NDT, 128, 2, 512).transpose(1, 2, 0, 3)).astype(bf16)
    bo = np.ascontiguousarray(np.asarray(out_b, np.float32)[None, :])

    tg = np.arange(NG) * GSTRIDE

    in_maps = []
    for c in range(NCORES):
        t0 = c * TLOC - HALO
        rows = np.arange(t0, t0 + NL)
        valid = (rows >= 0) & (rows < T)
        xlg = np.zeros((NPAD, D), np.float32)
        xlg[0:NL][valid] = x2[rows[valid]]
        xlg[NL:NLG] = x2[tg]
        xlg = xlg.astype(bf16)
        cs_rows = np.concatenate(
            [np.clip(rows, 0, T - 1), tg, np.zeros(NPAD - NLG, np.int64)])
        cl, sl = _cos_sin(cs_rows)                    # (NPAD, 32) each
        cl = np.tile(cl, (1, H))                      # (NPAD, 512) flat
        sl = np.tile(sl, (1, H))
        cslg = np.ascontiguousarray(
            np.concatenate([cl, sl], axis=1)).astype(bf16)

        ml = np.zeros((NSUP, 128, SUPW), np.float32)
        for m in range(NT):
            s, col = SUP[m]
            jj = (t0 + m * 128) + np.arange(128)
            ii = c * TLOC + QS[m] + np.arange(QW[m])
            ml[s, :, col:col + QW[m]] = np.where(
                _allowed(ii[None, :], jj[:, None]), 1.0, 0.0)
        iq = c * TLOC + np.arange(TLOC)
        qb = iq // 128
        jg = tg[:, None]
        covered = (jg >= 128 * (qb[None, :] - 1)) & (jg < 128 * (qb[None, :] + 2))
        mgl = np.where(covered, 0.0, 1.0).astype(np.float32)

        in_maps.append({
            "xlg": xlg, "cslg": cslg, "wts": wts, "wos": wos, "ball": ball,
            "bo": bo, "mloc": ml.astype(bf16), "mglob": mgl.astype(bf16),
            "identd": np.eye(128, dtype=bf16),
        })
    return in_maps


def kernel(x, in_proj_w, in_proj_b, out_w, out_b):
    from concourse.bass_utils import run_bass_kernel_spmd

    if "nc" not in _cache:
        _cache["nc"] = build_program()
    nc = _cache["nc"]
    in_maps = make_in_maps(x, in_proj_w, in_proj_b, out_w, out_b)
    res = run_bass_kernel_spmd(nc, in_maps, list(range(NCORES))).results
    pieces = [res[c]["out"] for c in range(NCORES)]
    return np.concatenate(pieces, axis=0).reshape(B, T, D).astype(np.float32)
